# revision 58
# baseline (speedup 1.0000x reference)
"""Trainium2 Bass kernel for nn_Atten_Block (non-local attention block).

Reference computation per batch element b (C=256, C4=64, H=W=64, N=4096):
    theta = W1 @ x + b1          [C4, N]
    phi   = W2 @ x + b2          [C4, N]
    g     = W3 @ x + b3          [C4, N]
    S     = theta^T @ phi        [N, N]
    A     = softmax(S, axis=-1)
    attn_g[c,i] = sum_j g[c,j] A[i,j]
    y     = x + W4 @ attn_g + b4

Sharding: data-parallel over batch B=8 across the 8 NeuronCores (one batch
element per core).

Per-core algorithm (engine-balanced around the ScalarE exp bottleneck):
  - S is computed TRANSPOSED: S^T tile [j=128, i=512] = phi_jblk.T @ theta_i
    so that softmax normalization and the PV matmul need no transposes:
      P^T = exp(S^T)  (no max-subtraction: |S| <= ~65 < 88, safe in fp32)
      pv[c,i] = sum_j gT[j,c] P^T[j,i]  via matmul with lhsT = [gT | ones]
    The appended ones column makes pv row 64 the softmax denominators l[i].
  - attn_g = pv[0:64] * (1/l) broadcast via a K=1 ones matmul.
  - y = x + W4 @ attn_g + b4 fused in one DVE op per tile.

Matmuls run in float32r (1 cyc/row vs fp32's 4) — producers round to f32r.
"""

import sys
from contextlib import ExitStack

import numpy as np

if "/opt/trn_rl_repo" not in sys.path:
    sys.path.insert(0, "/opt/trn_rl_repo")

C = 256
C4 = 64
B = 8
H = W = 64
N = H * W          # 4096
NI = 512           # i-tile width (matmul free dim)
NJ = 128           # j-block (S^T partition dim)
N_ITILES = N // NI   # 8
N_JBLKS = N // NJ    # 32

_CACHE = {}


def _build(cfg):
    import concourse.tile as tile
    from concourse import bacc, mybir

    F32 = mybir.dt.float32

    nc = bacc.Bacc("TRN2", target_bir_lowering=False, debug=False,
                   num_devices=B)

    F32R = mybir.dt.float32r
    MMD = F32R if cfg.get("f32r", True) else F32
    aps = dict(
        x_d=nc.dram_tensor("x", [128, 2 * N], MMD, kind="ExternalInput").ap(),
        w1_d=nc.dram_tensor("w1t", [128, 256], MMD, kind="ExternalInput").ap(),
        w2_d=nc.dram_tensor("w2t", [128, 256], MMD, kind="ExternalInput").ap(),
        w3_d=nc.dram_tensor("w3t", [128, 128], MMD, kind="ExternalInput").ap(),
        w4_d=nc.dram_tensor("w4t", [C4, C], MMD, kind="ExternalInput").ap(),
        b123_d=nc.dram_tensor("b123", [128, 3], F32, kind="ExternalInput").ap(),
        b4_d=nc.dram_tensor("b4c", [128, 2], F32, kind="ExternalInput").ap(),
        b3bc_d=nc.dram_tensor("b3bc", [128, C4], F32, kind="ExternalInput").ap(),
        y_d=nc.dram_tensor("y", [C, N], F32, kind="ExternalOutput").ap(),
    )

    with tile.TileContext(nc) as tc:
        _body(nc, tc, cfg, aps)
    nc.compile()
    return nc


def _body(nc, tc, cfg, aps):
    from concourse import mybir
    from concourse.alu_op_type import AluOpType as Alu

    F32 = mybir.dt.float32
    F32R = mybir.dt.float32r
    MM = F32R if cfg.get("f32r", True) else F32
    BF16 = mybir.dt.bfloat16
    PV = BF16 if cfg.get("pv_bf16", False) else MM
    Exp = mybir.ActivationFunctionType.Exp

    x_d, y_d = aps["x_d"], aps["y_d"]

    with ExitStack() as st:
        sb = st.enter_context(tc.tile_pool(name="sb", bufs=1))

        # ---- static SBUF tensors ----
        # x lands once as f32r (bit-identical to fp32 — DMA doesn't round);
        # matmuls read it as f32r, the residual adds bitcast it back to fp32.
        xr_sb = sb.tile([128, 2 * N], MM, tag="xr_sb")
        # theta/phi duplicated across both partition halves (rows 64-127 =
        # rows 0-63) so S^T matmul pairs can row-pack the full PE array.
        # per-n-tile tensors: dependency granularity lets the scheduler
        # slide early main-loop batches into phase-A DMA stalls
        th_t = [sb.tile([128, NI], MM, tag=f"th{n}", name=f"tht{n}")
                for n in range(N_ITILES)]
        ph_t = [sb.tile([128, NI], MM, tag=f"ph{n}", name=f"pht{n}")
                for n in range(N_ITILES)]
        gt_t = [sb.tile([128, 4 * (C4 + 1)], PV, tag=f"gt{n}",
                        name=f"gtt{n}") for n in range(N_ITILES)]

        def th_ap(i):
            return th_t[i][:]

        def ph_ap(jb):
            return ph_t[jb // 4][:, (jb % 4) * NJ:(jb % 4 + 1) * NJ]

        def gt_ap(jb):
            o = (jb % 4) * (C4 + 1)
            return gt_t[jb // 4][:, o:o + C4 + 1]
        w1_sb = sb.tile([128, 256], MM, tag="w1_sb")       # dup-M k-tiles
        w2_sb = sb.tile([128, 256], MM, tag="w2_sb")
        w3_sb = sb.tile([128, 128], MM, tag="w3_sb")
        w4_sb = sb.tile([C4, C], MM, tag="w4_sb")
        b123_sb = sb.tile([128, 3], F32, tag="b123_sb")
        b4_sb = sb.tile([128, 2], F32, tag="b4_sb")
        ones_sb = sb.tile([1, 128], F32, tag="ones_sb")
        ones_r_sb = sb.tile([1, C4], MM, tag="ones_r_sb")
        b3bc_sb = sb.tile([128, C4], F32, tag="b3bc_sb")

        # weights in — direct DMA to f32r tiles (dtype-matched, no cast)
        nc.sync.dma_start(w1_sb[:], aps["w1_d"][:])
        nc.sync.dma_start(w2_sb[:], aps["w2_d"][:])
        nc.sync.dma_start(w3_sb[:], aps["w3_d"][:])
        nc.sync.dma_start(w4_sb[:], aps["w4_d"][:])
        nc.sync.dma_start(b123_sb[:], aps["b123_d"][:])
        nc.sync.dma_start(b4_sb[:], aps["b4_d"][:])
        nc.sync.dma_start(b3bc_sb[:], aps["b3bc_d"][:])
        nc.vector.memset(ones_sb[:], 1.0)
        nc.vector.tensor_copy(ones_r_sb[:], ones_sb[:, 0:C4])
        ones_col = sb.tile([128, N_JBLKS], F32, tag="ones_col")
        nc.vector.memset(ones_col[:], 1.0)
        for n in range(N_ITILES):
            nc.vector.tensor_copy(
                gt_t[n][:].rearrange("p (j c) -> p j c", c=C4 + 1)
                [:, :, C4:C4 + 1],
                ones_col[:, 4 * n:4 * n + 4]
                .rearrange("p (j c) -> p j c", c=1))

        # x in per n-tile, both k halves, so the conv pipeline fills as
        # chunks land
        for n in range(N_ITILES):
            for k in range(2):
                c0 = k * N + n * NI
                nc.sync.dma_start(xr_sb[:, c0:c0 + NI], x_d[:, c0:c0 + NI])

        # ---- phase A: theta / phi conv1x1; gT direct ----
        with tc.tile_pool(name="psA", bufs=2, space="PSUM") as psA:
            # warm burst on the weight tiles (they land ~5us before the
            # first x chunk): a dense same-weight matmul run is the only
            # stream the HAM clock gate un-throttles on quickly; everything
            # after runs at 2.4GHz instead of 1.2.
            for r in range(22):
                wps = psA.tile([128, 256], F32, tag="warmps", name=f"warm{r}")
                nc.tensor.matmul(wps[:], w2_sb[:, 0:128], w1_sb[:],
                                 start=True, stop=True)

            def conv(dst_t, w_sb_, bias_col, m, n):
                ps = psA.tile([128, NI], F32, tag="convps")
                for k in range(2):
                    nc.tensor.matmul(
                        ps[0:m, :],
                        w_sb_[:, k * m:(k + 1) * m],
                        xr_sb[:, k * N + n * NI:k * N + (n + 1) * NI],
                        start=(k == 0), stop=(k == 1))
                # bias-add + PSUM->SBUF (+ f32r rounding) in one DVE op
                nc.vector.tensor_scalar_add(
                    dst_t[n][0:m, :], ps[0:m, :],
                    b123_sb[0:m, bias_col:bias_col + 1])

            def emit_gt(nb):
                tp = psA.tile([128, C4], F32, tag="tpps")
                for k in range(2):
                    nc.tensor.matmul(
                        tp[:],
                        xr_sb[:, k * N + nb * NJ:k * N + (nb + 1) * NJ],
                        w3_sb[:, k * C4:(k + 1) * C4],
                        start=(k == 0), stop=(k == 1))
                # bias-add (free-dim broadcast) + PSUM->SBUF in one DVE op
                o = (nb % 4) * (C4 + 1)
                nc.vector.scalar_tensor_tensor(
                    gt_t[nb // 4][:, o:o + C4], tp[:], 1.0,
                    b3bc_sb[:], Alu.mult, Alu.add)

            # per chunk-pair emission: everything for n-tile n is gated only
            # on x chunks (k0,n)/(k1,n), so work chases the DMA stream and
            # the first main-loop batches (which need only n=0 tiles) can
            # start while later chunks are still in flight.
            for n in range(N_ITILES):
                conv(ph_t, w2_sb, 1, 128, n)
                conv(th_t, w1_sb, 0, 128, n)
                for nb in range(4 * n, 4 * n + 4):
                    emit_gt(nb)

        # ---- main loop (PSUM: 4 stage + 2 pv + 1 misc + 1 warm = 8) ----
        SBANKS = cfg.get("stage_banks", 4)      # psum banks for S^T staging
        HALF = SBANKS // 2 * 512                # cols per staging half
        JPB = HALF // NI                        # j-blocks per exp batch
        ps_stage = st.enter_context(
            tc.tile_pool(name="ps_stage", bufs=3, space="PSUM"))
        ps_pv = st.enter_context(
            tc.tile_pool(name="ps_pv", bufs=2, space="PSUM"))
        pt_pool = st.enter_context(tc.tile_pool(name="pt", bufs=2))
        dv_pool = st.enter_context(tc.tile_pool(name="dv", bufs=2))
        y_pool = st.enter_context(tc.tile_pool(name="yp", bufs=3))

        # bf16 filler operands: bf16 matmuls get a separate prefetchable
        # LDWEIGHTS, so the per-filler cost is ~107ns
        wf_sb = sb.tile([128, 128], BF16, tag="wf_sb")
        nc.vector.tensor_copy(wf_sb[:], w2_sb[:, 0:128])
        xf_sb = sb.tile([128, 256], BF16, tag="xf_sb")
        nc.vector.tensor_copy(xf_sb[:], xr_sb[:, 0:256])

        def make_batches(i):
            # i == 0: start with single-j-block batches so the first exps
            # arrive quickly and the PE never idles a full HAM window while
            # the software pipeline fills.
            sizes = [1, 1] if i == 0 else []
            done = sum(sizes)
            while done < N_JBLKS:
                nb = min(JPB, N_JBLKS - done)
                sizes.append(nb)
                done += nb
            out, j = [], 0
            for s in sizes:
                out.append(list(range(j, j + s)))
                j += s
            return out

        pvs = [None] * N_ITILES

        def emit_s(i, b, batches):
            # row-packed pairs: even j-blocks on PE rows 0-63, odd on 64-127
            # (theta/phi are duplicated across halves) -> concurrent matmuls
            # and full-array activity for the HAM clock gate.
            stage_t = ps_stage.tile([128, HALF], F32, tag="stage",
                                    name=f"stage_{i}_{b}")
            # HAM filler: always-ready same-weight matmul into a region the
            # first S matmul below overwrites (start=True) — keeps the PE
            # from idling a throttle window, costs no extra PSUM bank.
            nc.tensor.matmul(stage_t[:, 0:256], wf_sb[:], xf_sb[:],
                             start=True, stop=True)
            half = stage_t[:, 0:len(batches[b]) * NI]
            for k, jb in enumerate(batches[b]):
                lo = (jb % 2) * C4
                nc.tensor.matmul(
                    half[:, k * NI:(k + 1) * NI],
                    ph_ap(jb)[lo:lo + C4, :],
                    th_t[i][lo:lo + C4, :],
                    start=True, stop=True,
                    tile_position=(lo, 0))
            return half

        def emit_exp_pv(i, b, half, batches):
            w = len(batches[b]) * NI
            pt = pt_pool.tile([128, HALF], PV, tag="pt")
            nc.scalar.activation(pt[:, 0:w], half[:], Exp)
            pv = pvs[i]
            for k, jb in enumerate(batches[b]):
                nc.tensor.matmul(
                    pv[0:C4 + 1, :],
                    gt_ap(jb),
                    pt[:, k * NI:(k + 1) * NI],
                    start=(jb == 0), stop=(jb == N_JBLKS - 1))

        tail_ag = {}

        def tail_recip(i):
            # DVE-only: 1/l as f32r, ready well before the bcast matmul
            pv = pvs[i]
            lrow = dv_pool.tile([1, NI], F32, tag="lrow")
            nc.vector.tensor_copy(lrow[:], pv[C4:C4 + 1, :])
            recip = dv_pool.tile([1, NI], F32, tag="recip")
            nc.vector.reciprocal_approx_fast(recip[:], lrow[:])
            recip_r = dv_pool.tile([1, NI], MM, tag="recip_r")
            nc.vector.tensor_copy(recip_r[:], recip[:])
            return recip_r

        def tail_bc(i, recip_r):
            pv = pvs[i]
            bc = ps_stage.tile([128, NI], F32, tag="stage", name=f"bc{i}")
            nc.tensor.matmul(bc[0:C4, :], ones_r_sb[:], recip_r[:],
                             start=True, stop=True)
            bcs = dv_pool.tile([C4, NI], F32, tag="bcs")
            nc.vector.tensor_copy(bcs[:], bc[0:C4, :])
            ag = dv_pool.tile([C4, NI], MM, tag="ag")
            nc.vector.tensor_tensor(ag[:], pv[0:C4, :], bcs[:], Alu.mult)
            return ag

        def tail_z(i, ag, h):
            z = ps_stage.tile([128, NI], F32, tag="stage", name=f"z{i}_{h}")
            nc.tensor.matmul(z[:], w4_sb[:, h * 128:(h + 1) * 128],
                             ag[:], start=True, stop=True)
            yt = y_pool.tile([128, NI], F32, tag="yt")
            # y = (z + b4) + x
            nc.vector.scalar_tensor_tensor(
                yt[:], z[:], b4_sb[:, h:h + 1],
                xr_sb[:, h * N + i * NI:h * N + (i + 1) * NI].bitcast(F32),
                Alu.add, Alu.add)
            nc.sync.dma_start(
                y_d[h * 128:(h + 1) * 128, i * NI:(i + 1) * NI], yt[:])

        # PE slack filler: the loop is ACT(exp)-bound with modest PE slack;
        # an always-ready same-weight dummy matmul per batch soaks up PE
        # bubbles so the HAM clock gate never sees an idle window (a single
        # ~1us PE gap re-throttles the PE to 1.2GHz for 15-30us).

        # software-pipelined emission: S(t+1) lands before exp/PV(t) on the
        # PE stream (cross-i-tile); the previous i-tile's tail chain is
        # spread over batches 3/5/8/11 of the next i-tile so the PE never
        # stalls behind the reciprocal chain.
        all_batches = {i: make_batches(i) for i in range(N_ITILES)}
        flat = [(i, b) for i in range(N_ITILES)
                for b in range(len(all_batches[i]))]
        halves = {}
        for i in range(N_ITILES):
            pvs[i] = ps_pv.tile([128, NI], F32, tag="pv", name=f"pv{i}")
        halves[flat[0]] = emit_s(*flat[0], all_batches[flat[0][0]])
        for t, (i, b) in enumerate(flat):
            if t + 1 < len(flat):
                ni, nb_ = flat[t + 1]
                halves[flat[t + 1]] = emit_s(ni, nb_, all_batches[ni])
            emit_exp_pv(i, b, halves.pop((i, b)), all_batches[i])
            if i > 0:
                if b == 3:
                    tail_ag["r"] = tail_recip(i - 1)
                elif b == 5:
                    tail_ag["ag"] = tail_bc(i - 1, tail_ag.pop("r"))
                elif b == 8:
                    tail_z(i - 1, tail_ag["ag"], 0)
                elif b == 11:
                    tail_z(i - 1, tail_ag.pop("ag"), 1)
        i = N_ITILES - 1
        ag_last = tail_bc(i, tail_recip(i))
        tail_z(i, ag_last, 0)
        tail_z(i, ag_last, 1)


def _prepare_core_inputs(x_b, W1, b1, W2, b2, W3, b3, W4, b4):
    def ktile(wT, m):
        # [256, m] -> [128, 2*m] (two k-tiles side by side)
        return np.ascontiguousarray(
            wT.reshape(2, 128, m).transpose(1, 0, 2).reshape(128, 2 * m))

    def dup(wT):
        # duplicate output channels across both halves: [256,64] -> [256,128]
        return np.concatenate([wT, wT], axis=1)

    z64 = np.zeros(C4, np.float32)
    return {
        "x": np.ascontiguousarray(
            x_b.reshape(2, 128, N).transpose(1, 0, 2).reshape(128, 2 * N)),
        "w1t": ktile(dup(W1.T), 128), "w2t": ktile(dup(W2.T), 128),
        "w3t": ktile(W3.T, C4),
        "w4t": np.ascontiguousarray(W4.T),
        "b123": np.ascontiguousarray(
            np.stack([np.r_[b1, b1], np.r_[b2, b2], np.r_[b3, z64]], axis=1)),
        "b4c": np.ascontiguousarray(b4.reshape(2, 128).T),
        "b3bc": np.ascontiguousarray(
            np.broadcast_to(b3.reshape(1, C4), (128, C4)).copy()),
    }


def kernel(x, W1, b1, W2, b2, W3, b3, W4, b4, _trace=False, _cfg=None):
    from concourse import bass_utils

    cfg = dict(_cfg or {})
    key = tuple(sorted(cfg.items()))
    if key not in _CACHE:
        _CACHE[key] = _build(cfg)
    nc = _CACHE[key]

    x = np.asarray(x, dtype=np.float32)
    xf = x.reshape(B, C, N)
    args = [np.asarray(a, dtype=np.float32)
            for a in (W1, b1, W2, b2, W3, b3, W4, b4)]
    in_maps = [_prepare_core_inputs(xf[b], *args) for b in range(B)]
    res = bass_utils.run_bass_kernel_spmd(
        nc, in_maps, core_ids=list(range(B)), trace=_trace)
    out = np.stack([res.results[b]["y"].reshape(C, H, W) for b in range(B)])
    if _trace:
        kernel.last_exec_time_ns = res.exec_time_ns
    return out


# revision 59
# speedup vs baseline: 1.2385x; 1.2385x over previous
"""Trainium2 Bass kernel for nn_Atten_Block (non-local attention block).

Reference computation per batch element b (C=256, C4=64, H=W=64, N=4096):
    theta = W1 @ x + b1          [C4, N]
    phi   = W2 @ x + b2          [C4, N]
    g     = W3 @ x + b3          [C4, N]
    S     = theta^T @ phi        [N, N]
    A     = softmax(S, axis=-1)
    attn_g[c,i] = sum_j g[c,j] A[i,j]
    y     = x + W4 @ attn_g + b4

Sharding: data-parallel over batch B=8 across the 8 NeuronCores (one batch
element per core).

Per-core algorithm (engine-balanced around the ScalarE exp bottleneck):
  - S is computed TRANSPOSED: S^T tile [j=128, i=512] = phi_jblk.T @ theta_i
    so that softmax normalization and the PV matmul need no transposes:
      P^T = exp(S^T)  (no max-subtraction: |S| <= ~65 < 88, safe in fp32)
      pv[c,i] = sum_j gT[j,c] P^T[j,i]  via matmul with lhsT = [gT | ones]
    The appended ones column makes pv row 64 the softmax denominators l[i].
  - attn_g = pv[0:64] * (1/l) broadcast via a K=1 ones matmul.
  - y = x + W4 @ attn_g + b4 fused in one DVE op per tile.

Matmuls run in float32r (1 cyc/row vs fp32's 4) — producers round to f32r.
"""

import sys
from contextlib import ExitStack

import numpy as np

if "/opt/trn_rl_repo" not in sys.path:
    sys.path.insert(0, "/opt/trn_rl_repo")

C = 256
C4 = 64
B = 8
H = W = 64
N = H * W          # 4096
NI = 512           # i-tile width (matmul free dim)
NJ = 128           # j-block (S^T partition dim)
N_ITILES = N // NI   # 8
N_JBLKS = N // NJ    # 32

_CACHE = {}


def _build(cfg):
    import concourse.tile as tile
    from concourse import bacc, mybir

    F32 = mybir.dt.float32

    nc = bacc.Bacc("TRN2", target_bir_lowering=False, debug=False,
                   num_devices=B)

    F32R = mybir.dt.float32r
    MMD = F32R if cfg.get("f32r", True) else F32
    aps = dict(
        x_d=nc.dram_tensor("x", [128, 2 * N], MMD, kind="ExternalInput").ap(),
        w1_d=nc.dram_tensor("w1t", [128, 256], MMD, kind="ExternalInput").ap(),
        w2_d=nc.dram_tensor("w2t", [128, 256], MMD, kind="ExternalInput").ap(),
        w3_d=nc.dram_tensor("w3t", [128, 128], MMD, kind="ExternalInput").ap(),
        w4_d=nc.dram_tensor("w4t", [C4, C], MMD, kind="ExternalInput").ap(),
        b123_d=nc.dram_tensor("b123", [128, 3], F32, kind="ExternalInput").ap(),
        b4_d=nc.dram_tensor("b4c", [128, 2], F32, kind="ExternalInput").ap(),
        b3bc_d=nc.dram_tensor("b3bc", [128, C4], F32, kind="ExternalInput").ap(),
        y_d=nc.dram_tensor("y", [C, N], F32, kind="ExternalOutput").ap(),
    )

    with tile.TileContext(nc) as tc:
        _body(nc, tc, cfg, aps)
    nc.compile()
    return nc


def _body(nc, tc, cfg, aps):
    from concourse import mybir
    from concourse.alu_op_type import AluOpType as Alu

    F32 = mybir.dt.float32
    F32R = mybir.dt.float32r
    MM = F32R if cfg.get("f32r", True) else F32
    BF16 = mybir.dt.bfloat16
    PV = BF16 if cfg.get("pv_bf16", False) else MM
    Exp = mybir.ActivationFunctionType.Exp

    x_d, y_d = aps["x_d"], aps["y_d"]

    with ExitStack() as st:
        sb = st.enter_context(tc.tile_pool(name="sb", bufs=1))

        # ---- static SBUF tensors ----
        # x lands once as f32r (bit-identical to fp32 — DMA doesn't round);
        # matmuls read it as f32r, the residual adds bitcast it back to fp32.
        xr_sb = sb.tile([128, 2 * N], MM, tag="xr_sb")
        # theta/phi duplicated across both partition halves (rows 64-127 =
        # rows 0-63) so S^T matmul pairs can row-pack the full PE array.
        # per-n-tile tensors: dependency granularity lets the scheduler
        # slide early main-loop batches into phase-A DMA stalls
        th_t = [sb.tile([128, NI], MM, tag=f"th{n}", name=f"tht{n}")
                for n in range(N_ITILES)]
        ph_t = [sb.tile([128, NI], MM, tag=f"ph{n}", name=f"pht{n}")
                for n in range(N_ITILES)]
        gt_t = [sb.tile([128, 4 * (C4 + 1)], PV, tag=f"gt{n}",
                        name=f"gtt{n}") for n in range(N_ITILES)]

        def th_ap(i):
            return th_t[i][:]

        def ph_ap(jb):
            return ph_t[jb // 4][:, (jb % 4) * NJ:(jb % 4 + 1) * NJ]

        def gt_ap(jb):
            o = (jb % 4) * (C4 + 1)
            return gt_t[jb // 4][:, o:o + C4 + 1]
        w1_sb = sb.tile([128, 256], MM, tag="w1_sb")       # dup-M k-tiles
        w2_sb = sb.tile([128, 256], MM, tag="w2_sb")
        w3_sb = sb.tile([128, 128], MM, tag="w3_sb")
        w4_sb = sb.tile([C4, C], MM, tag="w4_sb")
        b123_sb = sb.tile([128, 3], F32, tag="b123_sb")
        b4_sb = sb.tile([128, 2], F32, tag="b4_sb")
        ones_sb = sb.tile([1, 128], F32, tag="ones_sb")
        ones_r_sb = sb.tile([1, C4], MM, tag="ones_r_sb")
        b3bc_sb = sb.tile([128, C4], F32, tag="b3bc_sb")

        # weights in — direct DMA to f32r tiles (dtype-matched, no cast)
        nc.sync.dma_start(w1_sb[:], aps["w1_d"][:])
        nc.sync.dma_start(w2_sb[:], aps["w2_d"][:])
        nc.sync.dma_start(w3_sb[:], aps["w3_d"][:])
        nc.sync.dma_start(w4_sb[:], aps["w4_d"][:])
        nc.sync.dma_start(b123_sb[:], aps["b123_d"][:])
        nc.sync.dma_start(b4_sb[:], aps["b4_d"][:])
        nc.sync.dma_start(b3bc_sb[:], aps["b3bc_d"][:])
        nc.vector.memset(ones_sb[:], 1.0)
        nc.vector.tensor_copy(ones_r_sb[:], ones_sb[:, 0:C4])
        ones_col = sb.tile([128, N_JBLKS], F32, tag="ones_col")
        nc.vector.memset(ones_col[:], 1.0)
        for n in range(N_ITILES):
            nc.vector.tensor_copy(
                gt_t[n][:].rearrange("p (j c) -> p j c", c=C4 + 1)
                [:, :, C4:C4 + 1],
                ones_col[:, 4 * n:4 * n + 4]
                .rearrange("p (j c) -> p j c", c=1))

        # x in per n-tile, both k halves, so the conv pipeline fills as
        # chunks land
        for n in range(N_ITILES):
            for k in range(2):
                c0 = k * N + n * NI
                nc.sync.dma_start(xr_sb[:, c0:c0 + NI], x_d[:, c0:c0 + NI])

        # ---- phase A: theta / phi conv1x1; gT direct ----
        with tc.tile_pool(name="psA", bufs=2, space="PSUM") as psA:
            # warm burst on the weight tiles (they land ~5us before the
            # first x chunk): a dense same-weight matmul run is the only
            # stream the HAM clock gate un-throttles on quickly; everything
            # after runs at 2.4GHz instead of 1.2.
            for r in range(22):
                wps = psA.tile([128, 256], F32, tag="warmps", name=f"warm{r}")
                nc.tensor.matmul(wps[:], w2_sb[:, 0:128], w1_sb[:],
                                 start=True, stop=True)

            def conv(dst_t, w_sb_, bias_col, m, n):
                ps = psA.tile([128, NI], F32, tag="convps")
                for k in range(2):
                    nc.tensor.matmul(
                        ps[0:m, :],
                        w_sb_[:, k * m:(k + 1) * m],
                        xr_sb[:, k * N + n * NI:k * N + (n + 1) * NI],
                        start=(k == 0), stop=(k == 1))
                # bias-add + PSUM->SBUF (+ f32r rounding) in one DVE op
                nc.vector.tensor_scalar_add(
                    dst_t[n][0:m, :], ps[0:m, :],
                    b123_sb[0:m, bias_col:bias_col + 1])

            def emit_gt(nb):
                tp = psA.tile([128, C4], F32, tag="tpps")
                for k in range(2):
                    nc.tensor.matmul(
                        tp[:],
                        xr_sb[:, k * N + nb * NJ:k * N + (nb + 1) * NJ],
                        w3_sb[:, k * C4:(k + 1) * C4],
                        start=(k == 0), stop=(k == 1))
                # bias-add (free-dim broadcast) + PSUM->SBUF in one DVE op
                o = (nb % 4) * (C4 + 1)
                nc.vector.scalar_tensor_tensor(
                    gt_t[nb // 4][:, o:o + C4], tp[:], 1.0,
                    b3bc_sb[:], Alu.mult, Alu.add)

            # per chunk-pair emission: everything for n-tile n is gated only
            # on x chunks (k0,n)/(k1,n), so work chases the DMA stream and
            # the first main-loop batches (which need only n=0 tiles) can
            # start while later chunks are still in flight.
            for n in range(N_ITILES):
                conv(ph_t, w2_sb, 1, 128, n)
                conv(th_t, w1_sb, 0, 128, n)
                for nb in range(4 * n, 4 * n + 4):
                    emit_gt(nb)

        # ---- main loop (PSUM: 4 stage + 2 pv + 1 misc + 1 warm = 8) ----
        SBANKS = cfg.get("stage_banks", 4)      # psum banks for S^T staging
        HALF = SBANKS // 2 * 512                # cols per staging half
        JPB = HALF // NI                        # j-blocks per exp batch
        ps_stage = st.enter_context(
            tc.tile_pool(name="ps_stage", bufs=2, space="PSUM"))
        ps_pv = st.enter_context(
            tc.tile_pool(name="ps_pv", bufs=2, space="PSUM"))
        ps_misc = st.enter_context(
            tc.tile_pool(name="ps_misc", bufs=1, space="PSUM"))
        ps_warm = st.enter_context(
            tc.tile_pool(name="ps_warm", bufs=1, space="PSUM"))
        pt_pool = st.enter_context(tc.tile_pool(name="pt", bufs=2))
        dv_pool = st.enter_context(tc.tile_pool(name="dv", bufs=2))
        y_pool = st.enter_context(tc.tile_pool(name="yp", bufs=3))

        def make_batches(i):
            # i == 0: start with single-j-block batches so the first exps
            # arrive quickly and the PE never idles a full HAM window while
            # the software pipeline fills.
            sizes = [1, 1] if i == 0 else []
            done = sum(sizes)
            while done < N_JBLKS:
                nb = min(JPB, N_JBLKS - done)
                sizes.append(nb)
                done += nb
            out, j = [], 0
            for s in sizes:
                out.append(list(range(j, j + s)))
                j += s
            return out

        pvs = [None] * N_ITILES

        def emit_s(i, b, batches):
            # row-packed pairs: even j-blocks on PE rows 0-63, odd on 64-127
            # (theta/phi are duplicated across halves) -> concurrent matmuls
            # and full-array activity for the HAM clock gate.
            if i == 0 and b == 0:
                # borrow the misc bank so the PE has three S batches of
                # ready work while the first exps drain (pipeline fill)
                stage_t = ps_misc.tile([128, NI], F32, tag="misc",
                                       name="stage00")
            else:
                stage_t = ps_stage.tile([128, HALF], F32, tag="stage",
                                        name=f"stage_{i}_{b}")
            half = stage_t[:, 0:len(batches[b]) * NI]
            for k, jb in enumerate(batches[b]):
                lo = (jb % 2) * C4
                nc.tensor.matmul(
                    half[:, k * NI:(k + 1) * NI],
                    ph_ap(jb)[lo:lo + C4, :],
                    th_t[i][lo:lo + C4, :],
                    start=True, stop=True,
                    tile_position=(lo, 0))
            return half

        def emit_exp_pv(i, b, half, batches):
            w = len(batches[b]) * NI
            pt = pt_pool.tile([128, HALF], PV, tag="pt")
            nc.scalar.activation(pt[:, 0:w], half[:], Exp)
            pv = pvs[i]
            for k, jb in enumerate(batches[b]):
                nc.tensor.matmul(
                    pv[0:C4 + 1, :],
                    gt_ap(jb),
                    pt[:, k * NI:(k + 1) * NI],
                    start=(jb == 0), stop=(jb == N_JBLKS - 1))

        tail_ag = {}

        def tail_recip(i):
            # DVE-only: 1/l as f32r, ready well before the bcast matmul
            pv = pvs[i]
            lrow = dv_pool.tile([1, NI], F32, tag="lrow")
            nc.vector.tensor_copy(lrow[:], pv[C4:C4 + 1, :])
            recip = dv_pool.tile([1, NI], F32, tag="recip")
            nc.vector.reciprocal_approx_fast(recip[:], lrow[:])
            recip_r = dv_pool.tile([1, NI], MM, tag="recip_r")
            nc.vector.tensor_copy(recip_r[:], recip[:])
            return recip_r

        def tail_bc(i, recip_r):
            pv = pvs[i]
            bc = ps_misc.tile([128, NI], F32, tag="misc", name=f"bc{i}")
            nc.tensor.matmul(bc[0:C4, :], ones_r_sb[:], recip_r[:],
                             start=True, stop=True)
            bcs = dv_pool.tile([C4, NI], F32, tag="bcs")
            nc.vector.tensor_copy(bcs[:], bc[0:C4, :])
            ag = dv_pool.tile([C4, NI], MM, tag="ag")
            nc.vector.tensor_tensor(ag[:], pv[0:C4, :], bcs[:], Alu.mult)
            return ag

        def tail_z(i, ag, h):
            z = ps_misc.tile([128, NI], F32, tag="misc", name=f"z{i}_{h}")
            nc.tensor.matmul(z[:], w4_sb[:, h * 128:(h + 1) * 128],
                             ag[:], start=True, stop=True)
            yt = y_pool.tile([128, NI], F32, tag="yt")
            # y = (z + b4) + x
            nc.vector.scalar_tensor_tensor(
                yt[:], z[:], b4_sb[:, h:h + 1],
                xr_sb[:, h * N + i * NI:h * N + (i + 1) * NI].bitcast(F32),
                Alu.add, Alu.add)
            nc.sync.dma_start(
                y_d[h * 128:(h + 1) * 128, i * NI:(i + 1) * NI], yt[:])

        # PE slack filler: the loop is ACT(exp)-bound with modest PE slack;
        # an always-ready same-weight dummy matmul per batch soaks up PE
        # bubbles so the HAM clock gate never sees an idle window (a single
        # ~1us PE gap re-throttles the PE to 1.2GHz for 15-30us).
        warm_t = ps_warm.tile([128, NI], F32, tag="warm")
        # bf16 filler operands: bf16 matmuls get a separate prefetchable
        # LDWEIGHTS, so the per-filler cost is ~107ns instead of 214
        wf_sb = sb.tile([128, 128], BF16, tag="wf_sb")
        nc.vector.tensor_copy(wf_sb[:], w2_sb[:, 0:128])
        xf_sb = sb.tile([128, 256], BF16, tag="xf_sb")
        nc.vector.tensor_copy(xf_sb[:], xr_sb[:, 0:256])

        def emit_filler(t):
            nc.tensor.matmul(warm_t[:, 0:256], wf_sb[:], xf_sb[:],
                             start=True, stop=True)

        # software-pipelined emission: S(t+1) lands before exp/PV(t) on the
        # PE stream (cross-i-tile); the previous i-tile's tail chain is
        # spread over batches 3/5/8/11 of the next i-tile so the PE never
        # stalls behind the reciprocal chain.
        all_batches = {i: make_batches(i) for i in range(N_ITILES)}
        flat = [(i, b) for i in range(N_ITILES)
                for b in range(len(all_batches[i]))]
        halves = {}
        for i in range(N_ITILES):
            pvs[i] = ps_pv.tile([128, NI], F32, tag="pv", name=f"pv{i}")
        halves[flat[0]] = emit_s(*flat[0], all_batches[flat[0][0]])
        for t, (i, b) in enumerate(flat):
            if t + 1 < len(flat):
                ni, nb_ = flat[t + 1]
                halves[flat[t + 1]] = emit_s(ni, nb_, all_batches[ni])
            if b not in (5, 8, 11):
                emit_filler(t)
            emit_exp_pv(i, b, halves.pop((i, b)), all_batches[i])
            if i > 0:
                if b == 3:
                    tail_ag["r"] = tail_recip(i - 1)
                elif b == 5:
                    tail_ag["ag"] = tail_bc(i - 1, tail_ag.pop("r"))
                elif b == 8:
                    tail_z(i - 1, tail_ag["ag"], 0)
                elif b == 11:
                    tail_z(i - 1, tail_ag.pop("ag"), 1)
        i = N_ITILES - 1
        ag_last = tail_bc(i, tail_recip(i))
        tail_z(i, ag_last, 0)
        tail_z(i, ag_last, 1)


def _prepare_core_inputs(x_b, W1, b1, W2, b2, W3, b3, W4, b4):
    def ktile(wT, m):
        # [256, m] -> [128, 2*m] (two k-tiles side by side)
        return np.ascontiguousarray(
            wT.reshape(2, 128, m).transpose(1, 0, 2).reshape(128, 2 * m))

    def dup(wT):
        # duplicate output channels across both halves: [256,64] -> [256,128]
        return np.concatenate([wT, wT], axis=1)

    z64 = np.zeros(C4, np.float32)
    return {
        "x": np.ascontiguousarray(
            x_b.reshape(2, 128, N).transpose(1, 0, 2).reshape(128, 2 * N)),
        "w1t": ktile(dup(W1.T), 128), "w2t": ktile(dup(W2.T), 128),
        "w3t": ktile(W3.T, C4),
        "w4t": np.ascontiguousarray(W4.T),
        "b123": np.ascontiguousarray(
            np.stack([np.r_[b1, b1], np.r_[b2, b2], np.r_[b3, z64]], axis=1)),
        "b4c": np.ascontiguousarray(b4.reshape(2, 128).T),
        "b3bc": np.ascontiguousarray(
            np.broadcast_to(b3.reshape(1, C4), (128, C4)).copy()),
    }


def kernel(x, W1, b1, W2, b2, W3, b3, W4, b4, _trace=False, _cfg=None):
    from concourse import bass_utils

    cfg = dict(_cfg or {})
    key = tuple(sorted(cfg.items()))
    if key not in _CACHE:
        _CACHE[key] = _build(cfg)
    nc = _CACHE[key]

    x = np.asarray(x, dtype=np.float32)
    xf = x.reshape(B, C, N)
    args = [np.asarray(a, dtype=np.float32)
            for a in (W1, b1, W2, b2, W3, b3, W4, b4)]
    in_maps = [_prepare_core_inputs(xf[b], *args) for b in range(B)]
    res = bass_utils.run_bass_kernel_spmd(
        nc, in_maps, core_ids=list(range(B)), trace=_trace)
    out = np.stack([res.results[b]["y"].reshape(C, H, W) for b in range(B)])
    if _trace:
        kernel.last_exec_time_ns = res.exec_time_ns
    return out


# revision 62
# speedup vs baseline: 1.2398x; 1.0010x over previous
"""Trainium2 Bass kernel for nn_Atten_Block (non-local attention block).

Reference computation per batch element b (C=256, C4=64, H=W=64, N=4096):
    theta = W1 @ x + b1          [C4, N]
    phi   = W2 @ x + b2          [C4, N]
    g     = W3 @ x + b3          [C4, N]
    S     = theta^T @ phi        [N, N]
    A     = softmax(S, axis=-1)
    attn_g[c,i] = sum_j g[c,j] A[i,j]
    y     = x + W4 @ attn_g + b4

Sharding: data-parallel over batch B=8 across the 8 NeuronCores (one batch
element per core).

Per-core algorithm (engine-balanced around the ScalarE exp bottleneck):
  - S is computed TRANSPOSED: S^T tile [j=128, i=512] = phi_jblk.T @ theta_i
    so that softmax normalization and the PV matmul need no transposes:
      P^T = exp(S^T)  (no max-subtraction: |S| <= ~65 < 88, safe in fp32)
      pv[c,i] = sum_j gT[j,c] P^T[j,i]  via matmul with lhsT = [gT | ones]
    The appended ones column makes pv row 64 the softmax denominators l[i].
  - attn_g = pv[0:64] * (1/l) broadcast via a K=1 ones matmul.
  - y = x + W4 @ attn_g + b4 fused in one DVE op per tile.

Matmuls run in float32r (1 cyc/row vs fp32's 4) — producers round to f32r.
"""

import sys
from contextlib import ExitStack

import numpy as np

if "/opt/trn_rl_repo" not in sys.path:
    sys.path.insert(0, "/opt/trn_rl_repo")

C = 256
C4 = 64
B = 8
H = W = 64
N = H * W          # 4096
NI = 512           # i-tile width (matmul free dim)
NJ = 128           # j-block (S^T partition dim)
N_ITILES = N // NI   # 8
N_JBLKS = N // NJ    # 32

_CACHE = {}


def _build(cfg):
    import concourse.tile as tile
    from concourse import bacc, mybir

    F32 = mybir.dt.float32

    nc = bacc.Bacc("TRN2", target_bir_lowering=False, debug=False,
                   num_devices=B)

    F32R = mybir.dt.float32r
    MMD = F32R if cfg.get("f32r", True) else F32
    aps = dict(
        x_d=nc.dram_tensor("x", [128, 2 * N], MMD, kind="ExternalInput").ap(),
        w1_d=nc.dram_tensor("w1t", [128, 256], MMD, kind="ExternalInput").ap(),
        w2_d=nc.dram_tensor("w2t", [128, 256], MMD, kind="ExternalInput").ap(),
        w3_d=nc.dram_tensor("w3t", [128, 128], MMD, kind="ExternalInput").ap(),
        w4_d=nc.dram_tensor("w4t", [C4, C], MMD, kind="ExternalInput").ap(),
        b123_d=nc.dram_tensor("b123", [128, 3], F32, kind="ExternalInput").ap(),
        b4_d=nc.dram_tensor("b4c", [128, 2], F32, kind="ExternalInput").ap(),
        b3bc_d=nc.dram_tensor("b3bc", [128, C4], F32, kind="ExternalInput").ap(),
        y_d=nc.dram_tensor("y", [C, N], F32, kind="ExternalOutput").ap(),
    )

    with tile.TileContext(nc) as tc:
        _body(nc, tc, cfg, aps)
    nc.compile()
    return nc


def _body(nc, tc, cfg, aps):
    from concourse import mybir
    from concourse.alu_op_type import AluOpType as Alu

    F32 = mybir.dt.float32
    F32R = mybir.dt.float32r
    MM = F32R if cfg.get("f32r", True) else F32
    BF16 = mybir.dt.bfloat16
    PV = BF16 if cfg.get("pv_bf16", False) else MM
    Exp = mybir.ActivationFunctionType.Exp

    x_d, y_d = aps["x_d"], aps["y_d"]

    with ExitStack() as st:
        sb = st.enter_context(tc.tile_pool(name="sb", bufs=1))

        # ---- static SBUF tensors ----
        # x lands once as f32r (bit-identical to fp32 — DMA doesn't round);
        # matmuls read it as f32r, the residual adds bitcast it back to fp32.
        xr_sb = sb.tile([128, 2 * N], MM, tag="xr_sb")
        # theta/phi duplicated across both partition halves (rows 64-127 =
        # rows 0-63) so S^T matmul pairs can row-pack the full PE array.
        # per-n-tile tensors: dependency granularity lets the scheduler
        # slide early main-loop batches into phase-A DMA stalls
        th_t = [sb.tile([128, NI], MM, tag=f"th{n}", name=f"tht{n}")
                for n in range(N_ITILES)]
        ph_t = [sb.tile([128, NI], MM, tag=f"ph{n}", name=f"pht{n}")
                for n in range(N_ITILES)]
        gt_t = [sb.tile([128, 4 * (C4 + 1)], PV, tag=f"gt{n}",
                        name=f"gtt{n}") for n in range(N_ITILES)]

        def th_ap(i):
            return th_t[i][:]

        def ph_ap(jb):
            return ph_t[jb // 4][:, (jb % 4) * NJ:(jb % 4 + 1) * NJ]

        def gt_ap(jb):
            o = (jb % 4) * (C4 + 1)
            return gt_t[jb // 4][:, o:o + C4 + 1]
        w1_sb = sb.tile([128, 256], MM, tag="w1_sb")       # dup-M k-tiles
        w2_sb = sb.tile([128, 256], MM, tag="w2_sb")
        w3_sb = sb.tile([128, 128], MM, tag="w3_sb")
        w4_sb = sb.tile([C4, C], MM, tag="w4_sb")
        b123_sb = sb.tile([128, 3], F32, tag="b123_sb")
        b4_sb = sb.tile([128, 2], F32, tag="b4_sb")
        ones_sb = sb.tile([1, 128], F32, tag="ones_sb")
        ones_r_sb = sb.tile([1, C4], MM, tag="ones_r_sb")
        b3bc_sb = sb.tile([128, C4], F32, tag="b3bc_sb")

        # weights in — direct DMA to f32r tiles (dtype-matched, no cast)
        nc.sync.dma_start(w1_sb[:], aps["w1_d"][:])
        nc.sync.dma_start(w2_sb[:], aps["w2_d"][:])
        nc.sync.dma_start(w3_sb[:], aps["w3_d"][:])
        nc.sync.dma_start(w4_sb[:], aps["w4_d"][:])
        nc.sync.dma_start(b123_sb[:], aps["b123_d"][:])
        nc.sync.dma_start(b4_sb[:], aps["b4_d"][:])
        nc.sync.dma_start(b3bc_sb[:], aps["b3bc_d"][:])
        nc.vector.memset(ones_sb[:], 1.0)
        nc.vector.tensor_copy(ones_r_sb[:], ones_sb[:, 0:C4])
        ones_col = sb.tile([128, N_JBLKS], F32, tag="ones_col")
        nc.vector.memset(ones_col[:], 1.0)
        for n in range(N_ITILES):
            nc.vector.tensor_copy(
                gt_t[n][:].rearrange("p (j c) -> p j c", c=C4 + 1)
                [:, :, C4:C4 + 1],
                ones_col[:, 4 * n:4 * n + 4]
                .rearrange("p (j c) -> p j c", c=1))

        # x in per n-tile, both k halves, so the conv pipeline fills as
        # chunks land
        for n in range(N_ITILES):
            for k in range(2):
                c0 = k * N + n * NI
                nc.sync.dma_start(xr_sb[:, c0:c0 + NI], x_d[:, c0:c0 + NI])

        # ---- phase A: theta / phi conv1x1; gT direct ----
        with tc.tile_pool(name="psA", bufs=2, space="PSUM") as psA:

            def conv(dst_t, w_sb_, bias_col, m, n):
                ps = psA.tile([128, NI], F32, tag="convps")
                for k in range(2):
                    nc.tensor.matmul(
                        ps[0:m, :],
                        w_sb_[:, k * m:(k + 1) * m],
                        xr_sb[:, k * N + n * NI:k * N + (n + 1) * NI],
                        start=(k == 0), stop=(k == 1))
                # bias-add + PSUM->SBUF (+ f32r rounding) in one DVE op
                nc.vector.tensor_scalar_add(
                    dst_t[n][0:m, :], ps[0:m, :],
                    b123_sb[0:m, bias_col:bias_col + 1])

            def emit_gt(nb):
                tp = psA.tile([128, C4], F32, tag="tpps")
                for k in range(2):
                    nc.tensor.matmul(
                        tp[:],
                        xr_sb[:, k * N + nb * NJ:k * N + (nb + 1) * NJ],
                        w3_sb[:, k * C4:(k + 1) * C4],
                        start=(k == 0), stop=(k == 1))
                # bias-add (free-dim broadcast) + PSUM->SBUF in one DVE op
                o = (nb % 4) * (C4 + 1)
                nc.vector.scalar_tensor_tensor(
                    gt_t[nb // 4][:, o:o + C4], tp[:], 1.0,
                    b3bc_sb[:], Alu.mult, Alu.add)

            # n=0 work first: the first main-loop batches depend only on
            # these, and the ACT engine is not clock-gated — the exp
            # pipeline starts while the PE is still cold/warming.
            conv(ph_t, w2_sb, 1, 128, 0)
            conv(th_t, w1_sb, 0, 128, 0)
            for nb in range(4):
                emit_gt(nb)
            # warm burst on the weight tiles: a dense same-weight matmul
            # run is the only stream the HAM clock gate un-throttles on
            # quickly; everything after runs at 2.4GHz instead of 1.2.
            for r in range(22):
                wps = psA.tile([128, 256], F32, tag="warmps", name=f"warm{r}")
                nc.tensor.matmul(wps[:], w2_sb[:, 0:128], w1_sb[:],
                                 start=True, stop=True)
            # per chunk-pair emission: everything for n-tile n is gated only
            # on x chunks (k0,n)/(k1,n), so work chases the DMA stream and
            # the first main-loop batches (which need only n=0 tiles) can
            # start while later chunks are still in flight.
            for n in range(1, N_ITILES):
                conv(ph_t, w2_sb, 1, 128, n)
                conv(th_t, w1_sb, 0, 128, n)
                for nb in range(4 * n, 4 * n + 4):
                    emit_gt(nb)

        # ---- main loop (PSUM: 4 stage + 2 pv + 1 misc + 1 warm = 8) ----
        SBANKS = cfg.get("stage_banks", 4)      # psum banks for S^T staging
        HALF = SBANKS // 2 * 512                # cols per staging half
        JPB = HALF // NI                        # j-blocks per exp batch
        ps_stage = st.enter_context(
            tc.tile_pool(name="ps_stage", bufs=2, space="PSUM"))
        ps_pv = st.enter_context(
            tc.tile_pool(name="ps_pv", bufs=2, space="PSUM"))
        ps_misc = st.enter_context(
            tc.tile_pool(name="ps_misc", bufs=1, space="PSUM"))
        ps_warm = st.enter_context(
            tc.tile_pool(name="ps_warm", bufs=1, space="PSUM"))
        pt_pool = st.enter_context(tc.tile_pool(name="pt", bufs=2))
        dv_pool = st.enter_context(tc.tile_pool(name="dv", bufs=2))
        y_pool = st.enter_context(tc.tile_pool(name="yp", bufs=3))

        def make_batches(i):
            # i == 0: start with single-j-block batches so the first exps
            # arrive quickly and the PE never idles a full HAM window while
            # the software pipeline fills.
            sizes = [1, 1] if i == 0 else []
            done = sum(sizes)
            while done < N_JBLKS:
                nb = min(JPB, N_JBLKS - done)
                sizes.append(nb)
                done += nb
            out, j = [], 0
            for s in sizes:
                out.append(list(range(j, j + s)))
                j += s
            return out

        pvs = [None] * N_ITILES

        def emit_s(i, b, batches):
            # row-packed pairs: even j-blocks on PE rows 0-63, odd on 64-127
            # (theta/phi are duplicated across halves) -> concurrent matmuls
            # and full-array activity for the HAM clock gate.
            if i == 0 and b == 0:
                # borrow the misc bank so the PE has three S batches of
                # ready work while the first exps drain (pipeline fill)
                stage_t = ps_misc.tile([128, NI], F32, tag="misc",
                                       name="stage00")
            else:
                stage_t = ps_stage.tile([128, HALF], F32, tag="stage",
                                        name=f"stage_{i}_{b}")
            half = stage_t[:, 0:len(batches[b]) * NI]
            for k, jb in enumerate(batches[b]):
                lo = (jb % 2) * C4
                nc.tensor.matmul(
                    half[:, k * NI:(k + 1) * NI],
                    ph_ap(jb)[lo:lo + C4, :],
                    th_t[i][lo:lo + C4, :],
                    start=True, stop=True,
                    tile_position=(lo, 0))
            return half

        def emit_exp_pv(i, b, half, batches):
            w = len(batches[b]) * NI
            pt = pt_pool.tile([128, HALF], PV, tag="pt")
            nc.scalar.activation(pt[:, 0:w], half[:], Exp)
            pv = pvs[i]
            for k, jb in enumerate(batches[b]):
                nc.tensor.matmul(
                    pv[0:C4 + 1, :],
                    gt_ap(jb),
                    pt[:, k * NI:(k + 1) * NI],
                    start=(jb == 0), stop=(jb == N_JBLKS - 1))

        tail_ag = {}

        def tail_recip(i):
            # DVE-only: 1/l as f32r, ready well before the bcast matmul
            pv = pvs[i]
            lrow = dv_pool.tile([1, NI], F32, tag="lrow")
            nc.vector.tensor_copy(lrow[:], pv[C4:C4 + 1, :])
            recip = dv_pool.tile([1, NI], F32, tag="recip")
            nc.vector.reciprocal_approx_fast(recip[:], lrow[:])
            recip_r = dv_pool.tile([1, NI], MM, tag="recip_r")
            nc.vector.tensor_copy(recip_r[:], recip[:])
            return recip_r

        def tail_bc(i, recip_r):
            pv = pvs[i]
            bc = ps_misc.tile([128, NI], F32, tag="misc", name=f"bc{i}")
            nc.tensor.matmul(bc[0:C4, :], ones_r_sb[:], recip_r[:],
                             start=True, stop=True)
            bcs = dv_pool.tile([C4, NI], F32, tag="bcs")
            nc.vector.tensor_copy(bcs[:], bc[0:C4, :])
            ag = dv_pool.tile([C4, NI], MM, tag="ag")
            nc.vector.tensor_tensor(ag[:], pv[0:C4, :], bcs[:], Alu.mult)
            return ag

        def tail_z(i, ag, h):
            z = ps_misc.tile([128, NI], F32, tag="misc", name=f"z{i}_{h}")
            nc.tensor.matmul(z[:], w4_sb[:, h * 128:(h + 1) * 128],
                             ag[:], start=True, stop=True)
            yt = y_pool.tile([128, NI], F32, tag="yt")
            # y = (z + b4) + x
            nc.vector.scalar_tensor_tensor(
                yt[:], z[:], b4_sb[:, h:h + 1],
                xr_sb[:, h * N + i * NI:h * N + (i + 1) * NI].bitcast(F32),
                Alu.add, Alu.add)
            nc.sync.dma_start(
                y_d[h * 128:(h + 1) * 128, i * NI:(i + 1) * NI], yt[:])

        # PE slack filler: the loop is ACT(exp)-bound with modest PE slack;
        # an always-ready same-weight dummy matmul per batch soaks up PE
        # bubbles so the HAM clock gate never sees an idle window (a single
        # ~1us PE gap re-throttles the PE to 1.2GHz for 15-30us).
        warm_t = ps_warm.tile([128, NI], F32, tag="warm")
        # bf16 filler operands: bf16 matmuls get a separate prefetchable
        # LDWEIGHTS, so the per-filler cost is ~107ns instead of 214
        wf_sb = sb.tile([128, 128], BF16, tag="wf_sb")
        nc.vector.tensor_copy(wf_sb[:], w2_sb[:, 0:128])
        xf_sb = sb.tile([128, 256], BF16, tag="xf_sb")
        nc.vector.tensor_copy(xf_sb[:], xr_sb[:, 0:256])

        def emit_filler(t):
            nc.tensor.matmul(warm_t[:, 0:256], wf_sb[:], xf_sb[:],
                             start=True, stop=True)

        # software-pipelined emission: S(t+1) lands before exp/PV(t) on the
        # PE stream (cross-i-tile); the previous i-tile's tail chain is
        # spread over batches 3/5/8/11 of the next i-tile so the PE never
        # stalls behind the reciprocal chain.
        all_batches = {i: make_batches(i) for i in range(N_ITILES)}
        flat = [(i, b) for i in range(N_ITILES)
                for b in range(len(all_batches[i]))]
        halves = {}
        for i in range(N_ITILES):
            pvs[i] = ps_pv.tile([128, NI], F32, tag="pv", name=f"pv{i}")
        halves[flat[0]] = emit_s(*flat[0], all_batches[flat[0][0]])
        for t, (i, b) in enumerate(flat):
            if t + 1 < len(flat):
                ni, nb_ = flat[t + 1]
                halves[flat[t + 1]] = emit_s(ni, nb_, all_batches[ni])
            if b not in (5, 8, 11):
                emit_filler(t)
            emit_exp_pv(i, b, halves.pop((i, b)), all_batches[i])
            if i > 0:
                if b == 3:
                    tail_ag["r"] = tail_recip(i - 1)
                elif b == 5:
                    tail_ag["ag"] = tail_bc(i - 1, tail_ag.pop("r"))
                elif b == 8:
                    tail_z(i - 1, tail_ag["ag"], 0)
                elif b == 11:
                    tail_z(i - 1, tail_ag.pop("ag"), 1)
        i = N_ITILES - 1
        ag_last = tail_bc(i, tail_recip(i))
        tail_z(i, ag_last, 0)
        tail_z(i, ag_last, 1)


def _prepare_core_inputs(x_b, W1, b1, W2, b2, W3, b3, W4, b4):
    def ktile(wT, m):
        # [256, m] -> [128, 2*m] (two k-tiles side by side)
        return np.ascontiguousarray(
            wT.reshape(2, 128, m).transpose(1, 0, 2).reshape(128, 2 * m))

    def dup(wT):
        # duplicate output channels across both halves: [256,64] -> [256,128]
        return np.concatenate([wT, wT], axis=1)

    z64 = np.zeros(C4, np.float32)
    return {
        "x": np.ascontiguousarray(
            x_b.reshape(2, 128, N).transpose(1, 0, 2).reshape(128, 2 * N)),
        "w1t": ktile(dup(W1.T), 128), "w2t": ktile(dup(W2.T), 128),
        "w3t": ktile(W3.T, C4),
        "w4t": np.ascontiguousarray(W4.T),
        "b123": np.ascontiguousarray(
            np.stack([np.r_[b1, b1], np.r_[b2, b2], np.r_[b3, z64]], axis=1)),
        "b4c": np.ascontiguousarray(b4.reshape(2, 128).T),
        "b3bc": np.ascontiguousarray(
            np.broadcast_to(b3.reshape(1, C4), (128, C4)).copy()),
    }


def kernel(x, W1, b1, W2, b2, W3, b3, W4, b4, _trace=False, _cfg=None):
    from concourse import bass_utils

    cfg = dict(_cfg or {})
    key = tuple(sorted(cfg.items()))
    if key not in _CACHE:
        _CACHE[key] = _build(cfg)
    nc = _CACHE[key]

    x = np.asarray(x, dtype=np.float32)
    xf = x.reshape(B, C, N)
    args = [np.asarray(a, dtype=np.float32)
            for a in (W1, b1, W2, b2, W3, b3, W4, b4)]
    in_maps = [_prepare_core_inputs(xf[b], *args) for b in range(B)]
    res = bass_utils.run_bass_kernel_spmd(
        nc, in_maps, core_ids=list(range(B)), trace=_trace)
    out = np.stack([res.results[b]["y"].reshape(C, H, W) for b in range(B)])
    if _trace:
        kernel.last_exec_time_ns = res.exec_time_ns
    return out


# revision 63
# speedup vs baseline: 1.2428x; 1.0025x over previous
"""Trainium2 Bass kernel for nn_Atten_Block (non-local attention block).

Reference computation per batch element b (C=256, C4=64, H=W=64, N=4096):
    theta = W1 @ x + b1          [C4, N]
    phi   = W2 @ x + b2          [C4, N]
    g     = W3 @ x + b3          [C4, N]
    S     = theta^T @ phi        [N, N]
    A     = softmax(S, axis=-1)
    attn_g[c,i] = sum_j g[c,j] A[i,j]
    y     = x + W4 @ attn_g + b4

Sharding: data-parallel over batch B=8 across the 8 NeuronCores (one batch
element per core).

Per-core algorithm (engine-balanced around the ScalarE exp bottleneck):
  - S is computed TRANSPOSED: S^T tile [j=128, i=512] = phi_jblk.T @ theta_i
    so that softmax normalization and the PV matmul need no transposes:
      P^T = exp(S^T)  (no max-subtraction: |S| <= ~65 < 88, safe in fp32)
      pv[c,i] = sum_j gT[j,c] P^T[j,i]  via matmul with lhsT = [gT | ones]
    The appended ones column makes pv row 64 the softmax denominators l[i].
  - attn_g = pv[0:64] * (1/l) broadcast via a K=1 ones matmul.
  - y = x + W4 @ attn_g + b4 fused in one DVE op per tile.

Matmuls run in float32r (1 cyc/row vs fp32's 4) — producers round to f32r.
"""

import sys
from contextlib import ExitStack

import numpy as np

if "/opt/trn_rl_repo" not in sys.path:
    sys.path.insert(0, "/opt/trn_rl_repo")

C = 256
C4 = 64
B = 8
H = W = 64
N = H * W          # 4096
NI = 512           # i-tile width (matmul free dim)
NJ = 128           # j-block (S^T partition dim)
N_ITILES = N // NI   # 8
N_JBLKS = N // NJ    # 32

_CACHE = {}


def _build(cfg):
    import concourse.tile as tile
    from concourse import bacc, mybir

    F32 = mybir.dt.float32

    nc = bacc.Bacc("TRN2", target_bir_lowering=False, debug=False,
                   num_devices=B)

    F32R = mybir.dt.float32r
    MMD = F32R if cfg.get("f32r", True) else F32
    aps = dict(
        x_d=nc.dram_tensor("x", [128, 2 * N], MMD, kind="ExternalInput").ap(),
        w1_d=nc.dram_tensor("w1t", [128, 256], MMD, kind="ExternalInput").ap(),
        w2_d=nc.dram_tensor("w2t", [128, 256], MMD, kind="ExternalInput").ap(),
        w3_d=nc.dram_tensor("w3t", [128, 128], MMD, kind="ExternalInput").ap(),
        w4_d=nc.dram_tensor("w4t", [C4, C], MMD, kind="ExternalInput").ap(),
        b123_d=nc.dram_tensor("b123", [128, 3], F32, kind="ExternalInput").ap(),
        b4_d=nc.dram_tensor("b4c", [128, 2], F32, kind="ExternalInput").ap(),
        b3bc_d=nc.dram_tensor("b3bc", [128, C4], F32, kind="ExternalInput").ap(),
        y_d=nc.dram_tensor("y", [C, N], F32, kind="ExternalOutput").ap(),
    )

    with tile.TileContext(nc) as tc:
        _body(nc, tc, cfg, aps)
    nc.compile()
    return nc


def _body(nc, tc, cfg, aps):
    from concourse import mybir
    from concourse.alu_op_type import AluOpType as Alu

    F32 = mybir.dt.float32
    F32R = mybir.dt.float32r
    MM = F32R if cfg.get("f32r", True) else F32
    BF16 = mybir.dt.bfloat16
    PV = BF16 if cfg.get("pv_bf16", False) else MM
    Exp = mybir.ActivationFunctionType.Exp

    x_d, y_d = aps["x_d"], aps["y_d"]

    with ExitStack() as st:
        sb = st.enter_context(tc.tile_pool(name="sb", bufs=1))

        # ---- static SBUF tensors ----
        # x lands once as f32r (bit-identical to fp32 — DMA doesn't round);
        # matmuls read it as f32r, the residual adds bitcast it back to fp32.
        xr_sb = sb.tile([128, 2 * N], MM, tag="xr_sb")
        # theta/phi duplicated across both partition halves (rows 64-127 =
        # rows 0-63) so S^T matmul pairs can row-pack the full PE array.
        # per-n-tile tensors: dependency granularity lets the scheduler
        # slide early main-loop batches into phase-A DMA stalls
        th_t = [sb.tile([128, NI], MM, tag=f"th{n}", name=f"tht{n}")
                for n in range(N_ITILES)]
        ph_t = [sb.tile([128, NI], MM, tag=f"ph{n}", name=f"pht{n}")
                for n in range(N_ITILES)]
        gt_t = [sb.tile([128, 4 * (C4 + 1)], PV, tag=f"gt{n}",
                        name=f"gtt{n}") for n in range(N_ITILES)]

        def th_ap(i):
            return th_t[i][:]

        def ph_ap(jb):
            return ph_t[jb // 4][:, (jb % 4) * NJ:(jb % 4 + 1) * NJ]

        def gt_ap(jb):
            o = (jb % 4) * (C4 + 1)
            return gt_t[jb // 4][:, o:o + C4 + 1]
        w1_sb = sb.tile([128, 256], MM, tag="w1_sb")       # dup-M k-tiles
        w2_sb = sb.tile([128, 256], MM, tag="w2_sb")
        w3_sb = sb.tile([128, 128], MM, tag="w3_sb")
        w4_sb = sb.tile([C4, C], MM, tag="w4_sb")
        b123_sb = sb.tile([128, 3], F32, tag="b123_sb")
        b4_sb = sb.tile([128, 2], F32, tag="b4_sb")
        ones_sb = sb.tile([1, 128], F32, tag="ones_sb")
        ones_r_sb = sb.tile([1, C4], MM, tag="ones_r_sb")
        b3bc_sb = sb.tile([128, C4], F32, tag="b3bc_sb")

        # weights in — direct DMA to f32r tiles (dtype-matched, no cast)
        nc.sync.dma_start(w1_sb[:], aps["w1_d"][:])
        nc.sync.dma_start(w2_sb[:], aps["w2_d"][:])
        nc.sync.dma_start(w3_sb[:], aps["w3_d"][:])
        nc.sync.dma_start(w4_sb[:], aps["w4_d"][:])
        nc.sync.dma_start(b123_sb[:], aps["b123_d"][:])
        nc.sync.dma_start(b4_sb[:], aps["b4_d"][:])
        nc.sync.dma_start(b3bc_sb[:], aps["b3bc_d"][:])
        nc.vector.memset(ones_sb[:], 1.0)
        nc.vector.tensor_copy(ones_r_sb[:], ones_sb[:, 0:C4])
        ones_col = sb.tile([128, N_JBLKS], F32, tag="ones_col")
        nc.vector.memset(ones_col[:], 1.0)
        for n in range(N_ITILES):
            nc.vector.tensor_copy(
                gt_t[n][:].rearrange("p (j c) -> p j c", c=C4 + 1)
                [:, :, C4:C4 + 1],
                ones_col[:, 4 * n:4 * n + 4]
                .rearrange("p (j c) -> p j c", c=1))

        # x in per n-tile, both k halves, so the conv pipeline fills as
        # chunks land
        for n in range(N_ITILES):
            for k in range(2):
                c0 = k * N + n * NI
                nc.sync.dma_start(xr_sb[:, c0:c0 + NI], x_d[:, c0:c0 + NI])

        # ---- phase A: theta / phi conv1x1; gT direct ----
        with tc.tile_pool(name="psA", bufs=2, space="PSUM") as psA:

            def conv(dst_t, w_sb_, bias_col, m, n):
                ps = psA.tile([128, NI], F32, tag="convps")
                for k in range(2):
                    nc.tensor.matmul(
                        ps[0:m, :],
                        w_sb_[:, k * m:(k + 1) * m],
                        xr_sb[:, k * N + n * NI:k * N + (n + 1) * NI],
                        start=(k == 0), stop=(k == 1))
                # bias-add + PSUM->SBUF (+ f32r rounding) in one DVE op
                nc.vector.tensor_scalar_add(
                    dst_t[n][0:m, :], ps[0:m, :],
                    b123_sb[0:m, bias_col:bias_col + 1])

            def emit_gt(nb):
                tp = psA.tile([128, C4], F32, tag="tpps")
                for k in range(2):
                    nc.tensor.matmul(
                        tp[:],
                        xr_sb[:, k * N + nb * NJ:k * N + (nb + 1) * NJ],
                        w3_sb[:, k * C4:(k + 1) * C4],
                        start=(k == 0), stop=(k == 1))
                # bias-add (free-dim broadcast) + PSUM->SBUF in one DVE op
                o = (nb % 4) * (C4 + 1)
                nc.vector.scalar_tensor_tensor(
                    gt_t[nb // 4][:, o:o + C4], tp[:], 1.0,
                    b3bc_sb[:], Alu.mult, Alu.add)

            # n=0 work first: the first main-loop batches depend only on
            # these, and the ACT engine is not clock-gated — the exp
            # pipeline starts while the PE is still cold/warming.
            conv(ph_t, w2_sb, 1, 128, 0)
            conv(th_t, w1_sb, 0, 128, 0)
            for nb in range(4):
                emit_gt(nb)
            # warm burst on the weight tiles: a dense same-weight matmul
            # run is the only stream the HAM clock gate un-throttles on
            # quickly; everything after runs at 2.4GHz instead of 1.2.
            for r in range(22):
                wps = psA.tile([128, 256], F32, tag="warmps", name=f"warm{r}")
                nc.tensor.matmul(wps[:], w2_sb[:, 0:128], w1_sb[:],
                                 start=True, stop=True)
            # per chunk-pair emission: everything for n-tile n is gated only
            # on x chunks (k0,n)/(k1,n), so work chases the DMA stream and
            # the first main-loop batches (which need only n=0 tiles) can
            # start while later chunks are still in flight.
            for n in range(1, N_ITILES):
                conv(ph_t, w2_sb, 1, 128, n)
                conv(th_t, w1_sb, 0, 128, n)
                for nb in range(4 * n, 4 * n + 4):
                    emit_gt(nb)

        # ---- main loop (PSUM: 4 stage + 2 pv + 1 misc + 1 warm = 8) ----
        SBANKS = cfg.get("stage_banks", 4)      # psum banks for S^T staging
        HALF = SBANKS // 2 * 512                # cols per staging half
        JPB = HALF // NI                        # j-blocks per exp batch
        ps_stage = st.enter_context(
            tc.tile_pool(name="ps_stage", bufs=2, space="PSUM"))
        ps_pv = st.enter_context(
            tc.tile_pool(name="ps_pv", bufs=2, space="PSUM"))
        ps_misc = st.enter_context(
            tc.tile_pool(name="ps_misc", bufs=1, space="PSUM"))
        ps_warm = st.enter_context(
            tc.tile_pool(name="ps_warm", bufs=1, space="PSUM"))
        pt_pool = st.enter_context(tc.tile_pool(name="pt", bufs=2))
        dv_pool = st.enter_context(tc.tile_pool(name="dv", bufs=2))
        y_pool = st.enter_context(tc.tile_pool(name="yp", bufs=3))

        def make_batches(i):
            # i == 0: start with single-j-block batches so the first exps
            # arrive quickly and the PE never idles a full HAM window while
            # the software pipeline fills.
            sizes = [1, 1] if i == 0 else []
            done = sum(sizes)
            while done < N_JBLKS:
                nb = min(JPB, N_JBLKS - done)
                sizes.append(nb)
                done += nb
            out, j = [], 0
            for s in sizes:
                out.append(list(range(j, j + s)))
                j += s
            return out

        pvs = [None] * N_ITILES

        def emit_s(i, b, batches):
            # row-packed pairs: even j-blocks on PE rows 0-63, odd on 64-127
            # (theta/phi are duplicated across halves) -> concurrent matmuls
            # and full-array activity for the HAM clock gate.
            if i == 0 and b == 0:
                # borrow the misc bank so the PE has three S batches of
                # ready work while the first exps drain (pipeline fill)
                stage_t = ps_misc.tile([128, NI], F32, tag="misc",
                                       name="stage00")
            else:
                stage_t = ps_stage.tile([128, HALF], F32, tag="stage",
                                        name=f"stage_{i}_{b}")
            half = stage_t[:, 0:len(batches[b]) * NI]
            for k, jb in enumerate(batches[b]):
                lo = (jb % 2) * C4
                nc.tensor.matmul(
                    half[:, k * NI:(k + 1) * NI],
                    ph_ap(jb)[lo:lo + C4, :],
                    th_t[i][lo:lo + C4, :],
                    start=True, stop=True,
                    tile_position=(lo, 0))
            return half

        def emit_exp_pv(i, b, half, batches):
            w = len(batches[b]) * NI
            pt = pt_pool.tile([128, HALF], PV, tag="pt")
            nc.scalar.activation(pt[:, 0:w], half[:], Exp)
            pv = pvs[i]
            for k, jb in enumerate(batches[b]):
                nc.tensor.matmul(
                    pv[0:C4 + 1, :],
                    gt_ap(jb),
                    pt[:, k * NI:(k + 1) * NI],
                    start=(jb == 0), stop=(jb == N_JBLKS - 1))

        tail_ag = {}

        def tail_recip(i):
            # DVE-only: 1/l as f32r, ready well before the bcast matmul
            pv = pvs[i]
            lrow = dv_pool.tile([1, NI], F32, tag="lrow")
            nc.vector.tensor_copy(lrow[:], pv[C4:C4 + 1, :])
            recip = dv_pool.tile([1, NI], F32, tag="recip")
            nc.vector.reciprocal_approx_fast(recip[:], lrow[:])
            recip_r = dv_pool.tile([1, NI], MM, tag="recip_r")
            nc.vector.tensor_copy(recip_r[:], recip[:])
            return recip_r

        def tail_bc(i, recip_r):
            pv = pvs[i]
            bc = ps_misc.tile([128, NI], F32, tag="misc", name=f"bc{i}")
            nc.tensor.matmul(bc[0:C4, :], ones_r_sb[:], recip_r[:],
                             start=True, stop=True)
            bcs = dv_pool.tile([C4, NI], F32, tag="bcs")
            nc.vector.tensor_copy(bcs[:], bc[0:C4, :])
            ag = dv_pool.tile([C4, NI], MM, tag="ag")
            nc.vector.tensor_tensor(ag[:], pv[0:C4, :], bcs[:], Alu.mult)
            return ag

        def tail_z(i, ag, h):
            z = ps_misc.tile([128, NI], F32, tag="misc", name=f"z{i}_{h}")
            nc.tensor.matmul(z[:], w4_sb[:, h * 128:(h + 1) * 128],
                             ag[:], start=True, stop=True)
            yt = y_pool.tile([128, NI], F32, tag="yt")
            # y = (z + b4) + x
            nc.vector.scalar_tensor_tensor(
                yt[:], z[:], b4_sb[:, h:h + 1],
                xr_sb[:, h * N + i * NI:h * N + (i + 1) * NI].bitcast(F32),
                Alu.add, Alu.add)
            nc.sync.dma_start(
                y_d[h * 128:(h + 1) * 128, i * NI:(i + 1) * NI], yt[:])

        # PE slack filler: the loop is ACT(exp)-bound with modest PE slack;
        # an always-ready same-weight dummy matmul per batch soaks up PE
        # bubbles so the HAM clock gate never sees an idle window (a single
        # ~1us PE gap re-throttles the PE to 1.2GHz for 15-30us).
        warm_t = ps_warm.tile([128, NI], F32, tag="warm")
        # bf16 filler operands: bf16 matmuls get a separate prefetchable
        # LDWEIGHTS, so the per-filler cost is ~107ns instead of 214
        wf_sb = sb.tile([128, 128], BF16, tag="wf_sb")
        nc.vector.tensor_copy(wf_sb[:], w2_sb[:, 0:128])
        xf_sb = sb.tile([128, 256], BF16, tag="xf_sb")
        nc.vector.tensor_copy(xf_sb[:], xr_sb[:, 0:256])

        def emit_filler(t):
            nc.tensor.matmul(warm_t[:, 0:256], wf_sb[:], xf_sb[:],
                             start=True, stop=True)

        # software-pipelined emission: S(t+1) lands before exp/PV(t) on the
        # PE stream (cross-i-tile); the previous i-tile's tail chain is
        # spread over batches 3/5/8/11 of the next i-tile so the PE never
        # stalls behind the reciprocal chain.
        all_batches = {i: make_batches(i) for i in range(N_ITILES)}
        flat = [(i, b) for i in range(N_ITILES)
                for b in range(len(all_batches[i]))]
        halves = {}
        for i in range(N_ITILES):
            pvs[i] = ps_pv.tile([128, NI], F32, tag="pv", name=f"pv{i}")
        halves[flat[0]] = emit_s(*flat[0], all_batches[flat[0][0]])
        for t, (i, b) in enumerate(flat):
            if t + 1 < len(flat):
                ni, nb_ = flat[t + 1]
                halves[flat[t + 1]] = emit_s(ni, nb_, all_batches[ni])
            if b not in (5, 8, 11):
                emit_filler(t)
            emit_exp_pv(i, b, halves.pop((i, b)), all_batches[i])
            if i > 0:
                if b == 3:
                    tail_ag["r"] = tail_recip(i - 1)
                elif b == 5:
                    tail_ag["ag"] = tail_bc(i - 1, tail_ag.pop("r"))
                elif b == 8:
                    tail_z(i - 1, tail_ag["ag"], 0)
                elif b == 11:
                    tail_z(i - 1, tail_ag.pop("ag"), 1)
        # final tail: interleave fillers so the clock gate stays warm
        # through the last serial chain (it otherwise re-throttles right
        # after the last exp and the tail runs at half speed)
        i = N_ITILES - 1
        recip_last = tail_recip(i)
        emit_filler(-1)
        ag_last = tail_bc(i, recip_last)
        emit_filler(-2)
        tail_z(i, ag_last, 0)
        emit_filler(-3)
        tail_z(i, ag_last, 1)


def _prepare_core_inputs(x_b, W1, b1, W2, b2, W3, b3, W4, b4):
    def ktile(wT, m):
        # [256, m] -> [128, 2*m] (two k-tiles side by side)
        return np.ascontiguousarray(
            wT.reshape(2, 128, m).transpose(1, 0, 2).reshape(128, 2 * m))

    def dup(wT):
        # duplicate output channels across both halves: [256,64] -> [256,128]
        return np.concatenate([wT, wT], axis=1)

    z64 = np.zeros(C4, np.float32)
    return {
        "x": np.ascontiguousarray(
            x_b.reshape(2, 128, N).transpose(1, 0, 2).reshape(128, 2 * N)),
        "w1t": ktile(dup(W1.T), 128), "w2t": ktile(dup(W2.T), 128),
        "w3t": ktile(W3.T, C4),
        "w4t": np.ascontiguousarray(W4.T),
        "b123": np.ascontiguousarray(
            np.stack([np.r_[b1, b1], np.r_[b2, b2], np.r_[b3, z64]], axis=1)),
        "b4c": np.ascontiguousarray(b4.reshape(2, 128).T),
        "b3bc": np.ascontiguousarray(
            np.broadcast_to(b3.reshape(1, C4), (128, C4)).copy()),
    }


def kernel(x, W1, b1, W2, b2, W3, b3, W4, b4, _trace=False, _cfg=None):
    from concourse import bass_utils

    cfg = dict(_cfg or {})
    key = tuple(sorted(cfg.items()))
    if key not in _CACHE:
        _CACHE[key] = _build(cfg)
    nc = _CACHE[key]

    x = np.asarray(x, dtype=np.float32)
    xf = x.reshape(B, C, N)
    args = [np.asarray(a, dtype=np.float32)
            for a in (W1, b1, W2, b2, W3, b3, W4, b4)]
    in_maps = [_prepare_core_inputs(xf[b], *args) for b in range(B)]
    res = bass_utils.run_bass_kernel_spmd(
        nc, in_maps, core_ids=list(range(B)), trace=_trace)
    out = np.stack([res.results[b]["y"].reshape(C, H, W) for b in range(B)])
    if _trace:
        kernel.last_exec_time_ns = res.exec_time_ns
    return out
